# revision 4
# baseline (speedup 1.0000x reference)
"""ArcFace multi-core Bass kernel for TRN2 (8 NeuronCores).

Reference computation (see original nn module):
  kernel_norm = kernel / (||kernel||_col + 1e-6)
  cos = clip(emb @ kernel_norm, -1, 1)                       [B, C]
  output = cos.at[i, label[i]].set(cos_theta_m) * 64         [B, C]
  loss = mean(logsumexp(output, 1) - output[i, label[i]])
  returns (loss, output, cos)

Strategy: shard the class dim C=100000 across 8 cores (12500 each).
Each core computes its [512, 12500] slice of output=64*cos and cos, plus
per-row partial sums of exp(64*cos).  The margin fixup only touches the
512 label positions, so it (and the final logsumexp/loss) is done on the
host from the gathered tensors, with an O(B) correction of the exp-sums.
"""

import math
import os

import numpy as np

B, E, C = 512, 512, 100000
NCORES = 8
CS = C // NCORES  # 12500 columns per core
CT = 500          # column tile (one PSUM bank of fp32)
NCT = CS // CT    # 25
NB = B // 128     # 4 row chunks
NE = E // 128     # 4 contraction chunks

S_SCALE = 64.0
MARGIN = 0.5
COS_M = float(np.cos(MARGIN))
SIN_M = float(np.sin(MARGIN))
MM = float(np.sin(MARGIN) * MARGIN)
THRESHOLD = float(np.cos(np.pi - MARGIN))

_NC_CACHE = None


def _build_nc():
    """Build + compile the single-core Bass program (run SPMD on 8 cores)."""
    from contextlib import ExitStack

    import concourse.bass as bass
    import concourse.tile as tile
    from concourse import bacc, mybir

    f32 = mybir.dt.float32
    AF = mybir.ActivationFunctionType

    nc = bacc.Bacc("TRN2", target_bir_lowering=False, debug=False)

    embT_h = nc.dram_tensor("embT", [E, B], f32, kind="ExternalInput")
    ker_h = nc.dram_tensor("ker", [E, CS], f32, kind="ExternalInput")
    out_h = nc.dram_tensor("out", [B, CS], f32, kind="ExternalOutput")
    cos_h = nc.dram_tensor("cos", [B, CS], f32, kind="ExternalOutput")
    sums_h = nc.dram_tensor("sums", [128, NB], f32, kind="ExternalOutput")

    embT = embT_h.ap()
    # [p, e, c]: element (e*128+p, c) of the [E, CS] shard
    ker_r = ker_h.ap().rearrange("(e p) c -> p e c", p=128)
    out_r = out_h.ap().rearrange("(b p) c -> p b c", p=128)
    cos_r = cos_h.ap().rearrange("(b p) c -> p b c", p=128)

    with tile.TileContext(nc) as tc, ExitStack() as ctx:
        const_pool = ctx.enter_context(tc.tile_pool(name="const", bufs=1))
        emb_pool = ctx.enter_context(tc.tile_pool(name="emb", bufs=1))
        ker_pool = ctx.enter_context(tc.tile_pool(name="ker", bufs=3))
        sq_pool = ctx.enter_context(tc.tile_pool(name="sq", bufs=3))
        row_pool = ctx.enter_context(tc.tile_pool(name="row", bufs=2))
        bc_pool = ctx.enter_context(tc.tile_pool(name="bc", bufs=2))
        work_pool = ctx.enter_context(tc.tile_pool(name="work", bufs=3))
        red_pool = ctx.enter_context(tc.tile_pool(name="red", bufs=4))
        acc_pool = ctx.enter_context(tc.tile_pool(name="acc", bufs=1))
        mm_ps = ctx.enter_context(tc.tile_pool(name="mmps", bufs=3, space="PSUM"))
        ss_ps = ctx.enter_context(tc.tile_pool(name="ssps", bufs=2, space="PSUM"))
        bc_ps = ctx.enter_context(tc.tile_pool(name="bcps", bufs=2, space="PSUM"))

        ones_col = const_pool.tile([128, 1], f32, tag="ones_col")
        nc.vector.memset(ones_col[:], 1.0)
        ones_row = const_pool.tile([1, 128], f32, tag="ones_row")
        nc.vector.memset(ones_row[:], 1.0)
        ln_s = const_pool.tile([1, 1], f32, tag="ln_s")
        nc.vector.memset(ln_s[:], math.log(S_SCALE))

        acc = acc_pool.tile([128, NB], f32)
        nc.vector.memset(acc[:], 0.0)

        # embT chunks: emb_tiles[e] = [128, B] slice of rows e*128..e*128+127
        emb_tiles = []
        for e in range(NE):
            t = emb_pool.tile([128, B], f32, tag=f"embt{e}")
            nc.sync.dma_start(t[:], embT[e * 128:(e + 1) * 128, :])
            emb_tiles.append(t)

        for ct in range(NCT):
            csl = slice(ct * CT, (ct + 1) * CT)

            ktile = ker_pool.tile([128, NE, CT], f32, tag="ktile")
            nc.sync.dma_start(ktile[:], ker_r[:, :, csl])

            # column sum-of-squares, accumulated over the 4 E-chunks on PE
            ssp = ss_ps.tile([1, CT], f32, tag="ssp")
            for e in range(NE):
                sq = sq_pool.tile([128, CT], f32, tag="sq")
                if e % 2 == 0:
                    nc.scalar.activation(sq[:], ktile[:, e, :], AF.Square)
                else:
                    nc.vector.tensor_mul(sq[:], ktile[:, e, :], ktile[:, e, :])
                nc.tensor.matmul(
                    ssp[:], lhsT=ones_col[:], rhs=sq[:],
                    start=(e == 0), stop=(e == NE - 1),
                )

            # 64 / (sqrt(ss)) == exp(-0.5*ln(ss) + ln(64)); the reference's
            # +1e-6 on the norm is ~1e-7 relative here - far below fp32 noise.
            lnrow = row_pool.tile([1, CT], f32, tag="lnrow")
            nc.scalar.activation(lnrow[:], ssp[:], AF.Ln)
            r64row = row_pool.tile([1, CT], f32, tag="r64row")
            nc.scalar.activation(
                r64row[:], lnrow[:], AF.Exp, scale=-0.5, bias=ln_s[:]
            )

            # broadcast the per-column scale to all 128 partitions via rank-1
            # matmul: ones[1,128].T @ r64row[1,CT] -> [128, CT]
            bcp = bc_ps.tile([128, CT], f32, tag="bcp")
            nc.tensor.matmul(
                bcp[:], lhsT=ones_row[:], rhs=r64row[:], start=True, stop=True
            )
            bc64 = bc_pool.tile([128, CT], f32, tag="bc64")
            nc.vector.tensor_copy(bc64[:], bcp[:])

            out_ct = work_pool.tile([128, NB, CT], f32, tag="out_ct")
            cos_ct = work_pool.tile([128, NB, CT], f32, tag="cos_ct")
            for b in range(NB):
                mp = mm_ps.tile([128, CT], f32, tag="mp")
                for e in range(NE):
                    nc.tensor.matmul(
                        mp[:],
                        lhsT=emb_tiles[e][:, b * 128:(b + 1) * 128],
                        rhs=ktile[:, e, :],
                        start=(e == 0), stop=(e == NE - 1),
                    )
                # out = raw * (64/norm); cos = out/64; rowsum += sum(exp(out))
                nc.vector.tensor_mul(out_ct[:, b, :], mp[:], bc64[:])
                nc.vector.tensor_scalar_mul(
                    cos_ct[:, b, :], out_ct[:, b, :], 1.0 / S_SCALE
                )
                exp_t = work_pool.tile([128, CT], f32, tag="exp_t")
                red = red_pool.tile([128, 1], f32, tag="red")
                nc.scalar.activation(
                    exp_t[:], out_ct[:, b, :], AF.Exp, accum_out=red[:]
                )
                nc.vector.tensor_add(acc[:, b:b + 1], acc[:, b:b + 1], red[:])

            nc.sync.dma_start(out_r[:, :, csl], out_ct[:])
            nc.sync.dma_start(cos_r[:, :, csl], cos_ct[:])

        nc.sync.dma_start(sums_h.ap()[:], acc[:])

    nc.compile()
    return nc


def _get_nc():
    global _NC_CACHE
    if _NC_CACHE is None:
        _NC_CACHE = _build_nc()
    return _NC_CACHE


def _run_device(embT, ker, trace=False):
    """Run the SPMD kernel. Returns (out [B,C], cos [B,C], sumexp [B], results)."""
    from concourse.bass_utils import run_bass_kernel_spmd

    nc = _get_nc()
    in_maps = []
    for i in range(NCORES):
        shard = np.ascontiguousarray(ker[:, i * CS:(i + 1) * CS])
        in_maps.append({"embT": embT, "ker": shard})

    res = run_bass_kernel_spmd(
        nc, in_maps, core_ids=list(range(NCORES)), trace=trace
    )
    outs = res.results
    out = np.concatenate([outs[i]["out"] for i in range(NCORES)], axis=1)
    cos = np.concatenate([outs[i]["cos"] for i in range(NCORES)], axis=1)
    # sums[i][p, b] = sum_c exp(out[b*128+p, c]) over core i's columns
    sums = np.stack([outs[i]["sums"] for i in range(NCORES)]).sum(axis=0)
    sumexp = sums.T.reshape(-1)  # row r = b*128 + p
    return out, cos, sumexp, res


def kernel(embeddings, kernel, label):
    emb = np.ascontiguousarray(np.asarray(embeddings, dtype=np.float32))
    ker = np.ascontiguousarray(np.asarray(kernel, dtype=np.float32))
    lab = np.asarray(label).astype(np.int64)

    embT = np.ascontiguousarray(emb.T)
    out, cos, sumexp, _ = _run_device(embT, ker)

    # host-side margin fixup at the 512 label positions + loss
    idx = np.arange(B)
    cc = np.clip(cos[idx, lab].astype(np.float64), -1.0, 1.0)
    sin = np.sqrt(np.maximum(0.0, 1.0 - cc * cc))
    ctm = cc * COS_M - sin * SIN_M
    ctm = np.where(cc - THRESHOLD <= 0.0, cc - MM, ctm)
    new_logit = (S_SCALE * ctm).astype(np.float32)
    old_logit = out[idx, lab].copy()
    out[idx, lab] = new_logit

    se = (
        sumexp.astype(np.float64)
        - np.exp(old_logit.astype(np.float64))
        + np.exp(new_logit.astype(np.float64))
    )
    logZ = np.log(se)
    loss = np.float32(np.mean(logZ - new_logit.astype(np.float64)))
    return loss, out, cos


# revision 9
# speedup vs baseline: 1.9016x; 1.9016x over previous
"""ArcFace multi-core Bass kernel for TRN2 (8 NeuronCores).

Reference computation (see original nn module):
  kernel_norm = kernel / (||kernel||_col + 1e-6)
  cos = clip(emb @ kernel_norm, -1, 1)                       [B, C]
  output = cos.at[i, label[i]].set(cos_theta_m) * 64         [B, C]
  loss = mean(logsumexp(output, 1) - output[i, label[i]])
  returns (loss, output, cos)

Strategy: shard the class dim C=100000 across 8 cores (12500 each).
Each core computes its [512, 12500] slice of output=64*cos and cos, plus
per-row partial sums of exp(64*cos).  The margin fixup only touches the
512 label positions, so it (and the final logsumexp/loss) is done on the
host from the gathered tensors, with an O(B) correction of the exp-sums.

Device pipeline per 500-column tile:
  - casting DMA (SWDGE) loads the f32 kernel tile as bf16
  - square on DVE, column sum-of-squares via matmul with ones (PE)
  - 64/sqrt(ss) == Exp(-0.5*Ln(ss) + ln 64) on ACT; all activations are
    steered into ONE table set (natural_log_exp_and_others) to avoid
    per-tile ACT_TABLE_LOADs
  - per-column scale broadcast to 128 partitions via rank-1 matmul
  - 16 bf16 matmuls emb.T @ ker accumulate [128,500] fp32 PSUM tiles
  - epilogue: out = raw*scale (DVE), cos = out/64 (DVE/ACT split),
    exp row-sums fused into the ACT Exp via accum_out
  - 1 MB batched output DMAs
"""

import math
import os

import numpy as np

B, E, C = 512, 512, 100000
NCORES = 8
CS = C // NCORES  # 12500 columns per core
CT = 500          # column tile (one PSUM bank of fp32)
NCT = CS // CT    # 25
NB = B // 128     # 4 row chunks
NE = E // 128     # 4 contraction chunks

S_SCALE = 64.0
MARGIN = 0.5
COS_M = float(np.cos(MARGIN))
SIN_M = float(np.sin(MARGIN))
MM = float(np.sin(MARGIN) * MARGIN)
THRESHOLD = float(np.cos(np.pi - MARGIN))

_NC_CACHE = None


def _steer_act_tables():
    """Make the act-table chooser put Ln/Exp/Square in ONE set.

    bacc's insert_act_table_loads picks, per activation, the first set
    containing its function - Ln lands in 'natural_log' and Exp in
    'exp_and_others', causing an ACT_TABLE_LOAD ping-pong.  Claim these
    functions only in 'natural_log_exp_and_others' (which really does
    contain all of them) so every activation resolves to that one set.
    """
    from concourse import bacc, mybir

    if getattr(bacc, "_arcface_act_steered", False):
        return
    orig = bacc.get_activation_tables

    def patched(arch):
        tabs = orig(arch)
        combined = "natural_log_exp_and_others"
        steer = {
            mybir.ActivationFunctionType.Ln,
            mybir.ActivationFunctionType.Exp,
            mybir.ActivationFunctionType.Square,
        }
        if combined in tabs and steer <= tabs[combined]:
            for name in tabs:
                if name != combined:
                    tabs[name] = tabs[name] - steer
        return tabs

    bacc.get_activation_tables = patched
    bacc._arcface_act_steered = True


def _build_nc():
    """Build + compile the single-core Bass program (run SPMD on 8 cores)."""
    from contextlib import ExitStack

    import concourse.bass as bass
    import concourse.tile as tile
    from concourse import bacc, mybir

    _steer_act_tables()

    f32 = mybir.dt.float32
    bf16 = mybir.dt.bfloat16
    AF = mybir.ActivationFunctionType

    nc = bacc.Bacc("TRN2", target_bir_lowering=False, debug=False)

    embT_h = nc.dram_tensor("embT", [E, B], bf16, kind="ExternalInput")
    ker_h = nc.dram_tensor("ker", [E, CS], f32, kind="ExternalInput")
    out_h = nc.dram_tensor("out", [B, CS], f32, kind="ExternalOutput")
    cos_h = nc.dram_tensor("cos", [B, CS], f32, kind="ExternalOutput")
    sums_h = nc.dram_tensor("sums", [128, NB], f32, kind="ExternalOutput")

    embT = embT_h.ap()
    # [p, e, c]: element (e*128+p, c) of the [E, CS] shard
    ker_r = ker_h.ap().rearrange("(e p) c -> p e c", p=128)
    out_r = out_h.ap().rearrange("(b p) c -> p b c", p=128)
    cos_r = cos_h.ap().rearrange("(b p) c -> p b c", p=128)

    with tile.TileContext(nc) as tc, ExitStack() as ctx:
        const_pool = ctx.enter_context(tc.tile_pool(name="const", bufs=1))
        emb_pool = ctx.enter_context(tc.tile_pool(name="emb", bufs=1))
        kb_pool = ctx.enter_context(tc.tile_pool(name="kb", bufs=3))
        sq_pool = ctx.enter_context(tc.tile_pool(name="sq", bufs=2))
        row_pool = ctx.enter_context(tc.tile_pool(name="row", bufs=3))
        bc_pool = ctx.enter_context(tc.tile_pool(name="bc", bufs=2))
        work_pool = ctx.enter_context(tc.tile_pool(name="work", bufs=3))
        exp_pool = ctx.enter_context(tc.tile_pool(name="expp", bufs=3))
        red_pool = ctx.enter_context(tc.tile_pool(name="red", bufs=4))
        acc_pool = ctx.enter_context(tc.tile_pool(name="acc", bufs=1))
        mm_ps = ctx.enter_context(tc.tile_pool(name="mmps", bufs=3, space="PSUM"))
        ss_ps = ctx.enter_context(tc.tile_pool(name="ssps", bufs=2, space="PSUM"))
        bc_ps = ctx.enter_context(tc.tile_pool(name="bcps", bufs=2, space="PSUM"))

        ones_col = const_pool.tile([128, 1], bf16, tag="ones_col")
        nc.vector.memset(ones_col[:], 1.0)
        ones_row = const_pool.tile([1, 128], f32, tag="ones_row")
        nc.vector.memset(ones_row[:], 1.0)
        ln_s = const_pool.tile([1, 1], f32, tag="ln_s")
        nc.vector.memset(ln_s[:], math.log(S_SCALE))

        # embT chunks (bf16): emb_tiles[e] = rows e*128..e*128+127, all B cols
        emb_tiles = []
        for e in range(NE):
            t = emb_pool.tile([128, B], bf16, tag=f"embt{e}")
            nc.sync.dma_start(t[:], embT[e * 128:(e + 1) * 128, :])
            emb_tiles.append(t)

        acc = acc_pool.tile([128, NB], f32)
        nc.vector.memset(acc[:], 0.0)

        for ct in range(NCT):
            csl = slice(ct * CT, (ct + 1) * CT)

            # casting DMA: f32 in DRAM -> bf16 in SBUF (SWDGE/gpsimd only)
            kb = kb_pool.tile([128, NE, CT], bf16, tag="kb")
            nc.gpsimd.dma_start(out=kb[:], in_=ker_r[:, :, csl])

            # column sum-of-squares -> [1, CT] psum
            sq = sq_pool.tile([128, NE, CT], bf16, tag="sq")
            nc.vector.tensor_mul(sq[:], kb[:], kb[:])
            ssp = ss_ps.tile([1, CT], f32, tag="ssp")
            for e in range(NE):
                nc.tensor.matmul(
                    ssp[:], lhsT=ones_col[:], rhs=sq[:, e, :],
                    start=(e == 0), stop=(e == NE - 1),
                )

            # r64[c] = 64 / sqrt(ss[c]) = Exp(-0.5*Ln(ss) + ln 64)
            lnrow = row_pool.tile([1, CT], f32, tag="lnrow")
            nc.scalar.activation(lnrow[:], ssp[:], AF.Ln)
            r64row = row_pool.tile([1, CT], f32, tag="r64row")
            nc.scalar.activation(
                r64row[:], lnrow[:], AF.Exp, scale=-0.5, bias=ln_s[:]
            )

            # broadcast the per-column scale to all 128 partitions
            bcp = bc_ps.tile([128, CT], f32, tag="bcp")
            nc.tensor.matmul(
                bcp[:], lhsT=ones_row[:], rhs=r64row[:], start=True, stop=True
            )
            bc64 = bc_pool.tile([128, CT], f32, tag="bc64")
            nc.vector.tensor_copy(bc64[:], bcp[:])

            out_ct = work_pool.tile([128, NB, CT], f32, tag="out_ct")
            cos_ct = work_pool.tile([128, NB, CT], f32, tag="cos_ct")
            for b in range(NB):
                mp = mm_ps.tile([128, CT], f32, tag="mp")
                for e in range(NE):
                    nc.tensor.matmul(
                        mp[:],
                        lhsT=emb_tiles[e][:, b * 128:(b + 1) * 128],
                        rhs=kb[:, e, :],
                        start=(e == 0), stop=(e == NE - 1),
                    )
                # out = raw * (64/norm); cos = out/64; acc += sum exp(out)
                nc.vector.tensor_mul(out_ct[:, b, :], mp[:], bc64[:])
                if b % 2 == 0:
                    nc.vector.tensor_scalar_mul(
                        cos_ct[:, b, :], out_ct[:, b, :], 1.0 / S_SCALE
                    )
                else:
                    nc.scalar.mul(cos_ct[:, b, :], out_ct[:, b, :], 1.0 / S_SCALE)
                exp_t = exp_pool.tile([128, CT], f32, tag="exp_t")
                red = red_pool.tile([128, 1], f32, tag="red")
                nc.scalar.activation(
                    exp_t[:], out_ct[:, b, :], AF.Exp, accum_out=red[:]
                )
                nc.vector.tensor_add(acc[:, b:b + 1], acc[:, b:b + 1], red[:])

            nc.sync.dma_start(out_r[:, :, csl], out_ct[:])
            nc.sync.dma_start(cos_r[:, :, csl], cos_ct[:])

        nc.sync.dma_start(sums_h.ap()[:], acc[:])

    nc.compile()
    return nc


def _get_nc():
    global _NC_CACHE
    if _NC_CACHE is None:
        _NC_CACHE = _build_nc()
    return _NC_CACHE


def _run_device(embT, ker, trace=False):
    """Run the SPMD kernel. Returns (out [B,C], cos [B,C], sumexp [B], results)."""
    from concourse.bass_utils import run_bass_kernel_spmd

    nc = _get_nc()
    in_maps = []
    for i in range(NCORES):
        shard = np.ascontiguousarray(ker[:, i * CS:(i + 1) * CS])
        in_maps.append({"embT": embT, "ker": shard})

    res = run_bass_kernel_spmd(
        nc, in_maps, core_ids=list(range(NCORES)), trace=trace
    )
    outs = res.results
    out = np.concatenate([outs[i]["out"] for i in range(NCORES)], axis=1)
    cos = np.concatenate([outs[i]["cos"] for i in range(NCORES)], axis=1)
    # sums[i][p, b] = sum_c exp(out[b*128+p, c]) over core i's columns
    sums = np.stack([outs[i]["sums"] for i in range(NCORES)]).sum(axis=0)
    sumexp = sums.T.reshape(-1)  # row r = b*128 + p
    return out, cos, sumexp, res


def _embT_bf16(emb):
    import ml_dtypes

    return np.ascontiguousarray(emb.T.astype(ml_dtypes.bfloat16))


def kernel(embeddings, kernel, label):
    emb = np.ascontiguousarray(np.asarray(embeddings, dtype=np.float32))
    ker = np.ascontiguousarray(np.asarray(kernel, dtype=np.float32))
    lab = np.asarray(label).astype(np.int64)

    out, cos, sumexp, _ = _run_device(_embT_bf16(emb), ker)

    # host-side margin fixup at the 512 label positions + loss
    idx = np.arange(B)
    cc = np.clip(cos[idx, lab].astype(np.float64), -1.0, 1.0)
    sin = np.sqrt(np.maximum(0.0, 1.0 - cc * cc))
    ctm = cc * COS_M - sin * SIN_M
    ctm = np.where(cc - THRESHOLD <= 0.0, cc - MM, ctm)
    new_logit = (S_SCALE * ctm).astype(np.float32)
    old_logit = out[idx, lab].copy()
    out[idx, lab] = new_logit

    se = (
        sumexp.astype(np.float64)
        - np.exp(old_logit.astype(np.float64))
        + np.exp(new_logit.astype(np.float64))
    )
    logZ = np.log(se)
    loss = np.float32(np.mean(logZ - new_logit.astype(np.float64)))
    return loss, out, cos


# revision 11
# speedup vs baseline: 2.0113x; 1.0577x over previous
"""ArcFace multi-core Bass kernel for TRN2 (8 NeuronCores).

Reference computation (see original nn module):
  kernel_norm = kernel / (||kernel||_col + 1e-6)
  cos = clip(emb @ kernel_norm, -1, 1)                       [B, C]
  output = cos.at[i, label[i]].set(cos_theta_m) * 64         [B, C]
  loss = mean(logsumexp(output, 1) - output[i, label[i]])
  returns (loss, output, cos)

Strategy: shard the class dim C=100000 across 8 cores (12500 each).
Each core computes its [512, 12500] slice of output=64*cos and cos, plus
per-row partial sums of exp(64*cos).  The margin fixup only touches the
512 label positions, so it (and the final logsumexp/loss) is done on the
host from the gathered tensors, with an O(B) correction of the exp-sums.

Device pipeline per 500-column tile:
  - casting DMA (SWDGE) loads the f32 kernel tile as bf16
  - square on DVE, column sum-of-squares via matmul with ones (PE)
  - 64/sqrt(ss) == Exp(-0.5*Ln(ss) + ln 64) on ACT; all activations are
    steered into ONE table set (natural_log_exp_and_others) to avoid
    per-tile ACT_TABLE_LOADs
  - per-column scale broadcast to 128 partitions via rank-1 matmul
  - 16 bf16 matmuls emb.T @ ker accumulate [128,500] fp32 PSUM tiles
  - epilogue: out = raw*scale (DVE), cos = out/64 (DVE/ACT split),
    exp row-sums fused into the ACT Exp via accum_out
  - 1 MB batched output DMAs
"""

import math
import os

import numpy as np

B, E, C = 512, 512, 100000
NCORES = 8
CS = C // NCORES  # 12500 columns per core
CT = 500          # column tile (one PSUM bank of fp32)
NCT = CS // CT    # 25
NB = B // 128     # 4 row chunks
NE = E // 128     # 4 contraction chunks

S_SCALE = 64.0
MARGIN = 0.5
COS_M = float(np.cos(MARGIN))
SIN_M = float(np.sin(MARGIN))
MM = float(np.sin(MARGIN) * MARGIN)
THRESHOLD = float(np.cos(np.pi - MARGIN))

_NC_CACHE = None


def _steer_act_tables():
    """Make the act-table chooser put Ln/Exp/Square in ONE set.

    bacc's insert_act_table_loads picks, per activation, the first set
    containing its function - Ln lands in 'natural_log' and Exp in
    'exp_and_others', causing an ACT_TABLE_LOAD ping-pong.  Claim these
    functions only in 'natural_log_exp_and_others' (which really does
    contain all of them) so every activation resolves to that one set.
    """
    from concourse import bacc, mybir

    if getattr(bacc, "_arcface_act_steered", False):
        return
    orig = bacc.get_activation_tables

    def patched(arch):
        tabs = orig(arch)
        combined = "natural_log_exp_and_others"
        steer = {
            mybir.ActivationFunctionType.Ln,
            mybir.ActivationFunctionType.Exp,
            mybir.ActivationFunctionType.Square,
        }
        if combined in tabs and steer <= tabs[combined]:
            for name in tabs:
                if name != combined:
                    tabs[name] = tabs[name] - steer
        return tabs

    bacc.get_activation_tables = patched
    bacc._arcface_act_steered = True


def _build_nc():
    """Build + compile the single-core Bass program (run SPMD on 8 cores)."""
    from contextlib import ExitStack

    import concourse.bass as bass
    import concourse.tile as tile
    from concourse import bacc, mybir

    _steer_act_tables()

    f32 = mybir.dt.float32
    bf16 = mybir.dt.bfloat16
    AF = mybir.ActivationFunctionType

    nc = bacc.Bacc("TRN2", target_bir_lowering=False, debug=False)

    embT_h = nc.dram_tensor("embT", [E, B], bf16, kind="ExternalInput")
    ker_h = nc.dram_tensor("ker", [E, CS], f32, kind="ExternalInput")
    out_h = nc.dram_tensor("out", [B, CS], f32, kind="ExternalOutput")
    cos_h = nc.dram_tensor("cos", [B, CS], f32, kind="ExternalOutput")
    sums_h = nc.dram_tensor("sums", [128, NB], f32, kind="ExternalOutput")

    embT = embT_h.ap()
    # [p, e, c]: element (e*128+p, c) of the [E, CS] shard
    ker_r = ker_h.ap().rearrange("(e p) c -> p e c", p=128)
    out_r = out_h.ap().rearrange("(b p) c -> p b c", p=128)
    cos_r = cos_h.ap().rearrange("(b p) c -> p b c", p=128)

    with tile.TileContext(nc) as tc, ExitStack() as ctx:
        const_pool = ctx.enter_context(tc.tile_pool(name="const", bufs=1))
        emb_pool = ctx.enter_context(tc.tile_pool(name="emb", bufs=1))
        kb_pool = ctx.enter_context(tc.tile_pool(name="kb", bufs=4))
        sq_pool = ctx.enter_context(tc.tile_pool(name="sq", bufs=2))
        row_pool = ctx.enter_context(tc.tile_pool(name="row", bufs=3))
        bc_pool = ctx.enter_context(tc.tile_pool(name="bc", bufs=2))
        work_pool = ctx.enter_context(tc.tile_pool(name="work", bufs=3))
        exp_pool = ctx.enter_context(tc.tile_pool(name="expp", bufs=3))
        red_pool = ctx.enter_context(tc.tile_pool(name="red", bufs=4))
        acc_pool = ctx.enter_context(tc.tile_pool(name="acc", bufs=1))
        mm_ps = ctx.enter_context(tc.tile_pool(name="mmps", bufs=4, space="PSUM"))
        ss_ps = ctx.enter_context(tc.tile_pool(name="ssps", bufs=2, space="PSUM"))

        ones_col = const_pool.tile([128, 1], bf16, tag="ones_col")
        nc.vector.memset(ones_col[:], 1.0)
        ln_s = const_pool.tile([1, 1], f32, tag="ln_s")
        nc.vector.memset(ln_s[:], math.log(S_SCALE))

        # embT chunks (bf16): emb_tiles[e] = rows e*128..e*128+127, all B cols
        emb_tiles = []
        for e in range(NE):
            t = emb_pool.tile([128, B], bf16, tag=f"embt{e}")
            nc.sync.dma_start(t[:], embT[e * 128:(e + 1) * 128, :])
            emb_tiles.append(t)

        acc = acc_pool.tile([128, NB], f32)
        nc.vector.memset(acc[:], 0.0)

        for ct in range(NCT):
            csl = slice(ct * CT, (ct + 1) * CT)

            # casting DMA: f32 in DRAM -> bf16 in SBUF (SWDGE/gpsimd only)
            kb = kb_pool.tile([128, NE, CT], bf16, tag="kb")
            nc.gpsimd.dma_start(out=kb[:], in_=ker_r[:, :, csl])

            # column sum-of-squares -> [1, CT] psum
            sq = sq_pool.tile([128, NE, CT], bf16, tag="sq")
            nc.vector.tensor_mul(sq[:], kb[:], kb[:])
            ssp = ss_ps.tile([1, CT], f32, tag="ssp")
            for e in range(NE):
                nc.tensor.matmul(
                    ssp[:], lhsT=ones_col[:], rhs=sq[:, e, :],
                    start=(e == 0), stop=(e == NE - 1),
                )

            # r64[c] = 64 / sqrt(ss[c]) = Exp(-0.5*Ln(ss) + ln 64)
            lnrow = row_pool.tile([1, CT], f32, tag="lnrow")
            nc.scalar.activation(lnrow[:], ssp[:], AF.Ln)
            r64row = row_pool.tile([1, CT], f32, tag="r64row")
            nc.scalar.activation(
                r64row[:], lnrow[:], AF.Exp, scale=-0.5, bias=ln_s[:]
            )

            # broadcast the per-column scale to all 128 partitions (GpSimd
            # is otherwise idle; PE rank-1 fp32 matmul costs 4x a bf16 one)
            bc64 = bc_pool.tile([128, CT], f32, tag="bc64")
            nc.gpsimd.partition_broadcast(bc64[:], r64row[:])

            out_ct = work_pool.tile([128, NB, CT], f32, tag="out_ct")
            cos_ct = work_pool.tile([128, NB, CT], f32, tag="cos_ct")
            for b in range(NB):
                mp = mm_ps.tile([128, CT], f32, tag="mp")
                for e in range(NE):
                    nc.tensor.matmul(
                        mp[:],
                        lhsT=emb_tiles[e][:, b * 128:(b + 1) * 128],
                        rhs=kb[:, e, :],
                        start=(e == 0), stop=(e == NE - 1),
                    )
                # out = raw * (64/norm); cos = out/64; acc += sum exp(out)
                nc.vector.tensor_mul(out_ct[:, b, :], mp[:], bc64[:])
                if b % 2 == 0:
                    nc.vector.tensor_scalar_mul(
                        cos_ct[:, b, :], out_ct[:, b, :], 1.0 / S_SCALE
                    )
                else:
                    nc.scalar.mul(cos_ct[:, b, :], out_ct[:, b, :], 1.0 / S_SCALE)
                exp_t = exp_pool.tile([128, CT], f32, tag="exp_t")
                red = red_pool.tile([128, 1], f32, tag="red")
                nc.scalar.activation(
                    exp_t[:], out_ct[:, b, :], AF.Exp, accum_out=red[:]
                )
                nc.vector.tensor_add(acc[:, b:b + 1], acc[:, b:b + 1], red[:])

            nc.sync.dma_start(out_r[:, :, csl], out_ct[:])
            nc.sync.dma_start(cos_r[:, :, csl], cos_ct[:])

        nc.sync.dma_start(sums_h.ap()[:], acc[:])

    nc.compile()
    return nc


def _get_nc():
    global _NC_CACHE
    if _NC_CACHE is None:
        _NC_CACHE = _build_nc()
    return _NC_CACHE


def _run_device(embT, ker, trace=False):
    """Run the SPMD kernel. Returns (out [B,C], cos [B,C], sumexp [B], results)."""
    from concourse.bass_utils import run_bass_kernel_spmd

    nc = _get_nc()
    in_maps = []
    for i in range(NCORES):
        shard = np.ascontiguousarray(ker[:, i * CS:(i + 1) * CS])
        in_maps.append({"embT": embT, "ker": shard})

    res = run_bass_kernel_spmd(
        nc, in_maps, core_ids=list(range(NCORES)), trace=trace
    )
    outs = res.results
    out = np.concatenate([outs[i]["out"] for i in range(NCORES)], axis=1)
    cos = np.concatenate([outs[i]["cos"] for i in range(NCORES)], axis=1)
    # sums[i][p, b] = sum_c exp(out[b*128+p, c]) over core i's columns
    sums = np.stack([outs[i]["sums"] for i in range(NCORES)]).sum(axis=0)
    sumexp = sums.T.reshape(-1)  # row r = b*128 + p
    return out, cos, sumexp, res


def _embT_bf16(emb):
    import ml_dtypes

    return np.ascontiguousarray(emb.T.astype(ml_dtypes.bfloat16))


def kernel(embeddings, kernel, label):
    emb = np.ascontiguousarray(np.asarray(embeddings, dtype=np.float32))
    ker = np.ascontiguousarray(np.asarray(kernel, dtype=np.float32))
    lab = np.asarray(label).astype(np.int64)

    out, cos, sumexp, _ = _run_device(_embT_bf16(emb), ker)

    # host-side margin fixup at the 512 label positions + loss
    idx = np.arange(B)
    cc = np.clip(cos[idx, lab].astype(np.float64), -1.0, 1.0)
    sin = np.sqrt(np.maximum(0.0, 1.0 - cc * cc))
    ctm = cc * COS_M - sin * SIN_M
    ctm = np.where(cc - THRESHOLD <= 0.0, cc - MM, ctm)
    new_logit = (S_SCALE * ctm).astype(np.float32)
    old_logit = out[idx, lab].copy()
    out[idx, lab] = new_logit

    se = (
        sumexp.astype(np.float64)
        - np.exp(old_logit.astype(np.float64))
        + np.exp(new_logit.astype(np.float64))
    )
    logZ = np.log(se)
    loss = np.float32(np.mean(logZ - new_logit.astype(np.float64)))
    return loss, out, cos


# revision 12
# speedup vs baseline: 2.2077x; 1.0976x over previous
"""ArcFace multi-core Bass kernel for TRN2 (8 NeuronCores).

Reference computation (see original nn module):
  kernel_norm = kernel / (||kernel||_col + 1e-6)
  cos = clip(emb @ kernel_norm, -1, 1)                       [B, C]
  output = cos.at[i, label[i]].set(cos_theta_m) * 64         [B, C]
  loss = mean(logsumexp(output, 1) - output[i, label[i]])
  returns (loss, output, cos)

Strategy: shard the class dim C=100000 across 8 cores (12500 each).
Each core computes its [512, 12500] slice of output=64*cos and cos, plus
per-row partial sums of exp(64*cos).  The margin fixup only touches the
512 label positions, so it (and the final logsumexp/loss) is done on the
host from the gathered tensors, with an O(B) correction of the exp-sums.

Device pipeline per 500-column tile:
  - casting DMA (SWDGE) loads the f32 kernel tile as bf16
  - square on DVE, column sum-of-squares via matmul with ones (PE)
  - 64/sqrt(ss) == Exp(-0.5*Ln(ss) + ln 64) on ACT; all activations are
    steered into ONE table set (natural_log_exp_and_others) to avoid
    per-tile ACT_TABLE_LOADs
  - per-column scale broadcast to 128 partitions via rank-1 matmul
  - 16 bf16 matmuls emb.T @ ker accumulate [128,500] fp32 PSUM tiles
  - epilogue: out = raw*scale (DVE), cos = out/64 (DVE/ACT split),
    exp row-sums fused into the ACT Exp via accum_out
  - 1 MB batched output DMAs
"""

import math
import os

import numpy as np

B, E, C = 512, 512, 100000
NCORES = 8
CS = C // NCORES  # 12500 columns per core
CT = 500          # column tile (one PSUM bank of fp32)
NCT = CS // CT    # 25
NB = B // 128     # 4 row chunks
NE = E // 128     # 4 contraction chunks

S_SCALE = 64.0
MARGIN = 0.5
COS_M = float(np.cos(MARGIN))
SIN_M = float(np.sin(MARGIN))
MM = float(np.sin(MARGIN) * MARGIN)
THRESHOLD = float(np.cos(np.pi - MARGIN))

_NC_CACHE = None


def _steer_act_tables():
    """Make the act-table chooser put Ln/Exp/Square in ONE set.

    bacc's insert_act_table_loads picks, per activation, the first set
    containing its function - Ln lands in 'natural_log' and Exp in
    'exp_and_others', causing an ACT_TABLE_LOAD ping-pong.  Claim these
    functions only in 'natural_log_exp_and_others' (which really does
    contain all of them) so every activation resolves to that one set.
    """
    from concourse import bacc, mybir

    if getattr(bacc, "_arcface_act_steered", False):
        return
    orig = bacc.get_activation_tables

    def patched(arch):
        tabs = orig(arch)
        combined = "natural_log_exp_and_others"
        steer = {
            mybir.ActivationFunctionType.Ln,
            mybir.ActivationFunctionType.Exp,
            mybir.ActivationFunctionType.Square,
        }
        if combined in tabs and steer <= tabs[combined]:
            for name in tabs:
                if name != combined:
                    tabs[name] = tabs[name] - steer
        return tabs

    bacc.get_activation_tables = patched
    bacc._arcface_act_steered = True


def _build_nc():
    """Build + compile the single-core Bass program (run SPMD on 8 cores)."""
    from contextlib import ExitStack

    import concourse.bass as bass
    import concourse.tile as tile
    from concourse import bacc, mybir

    _steer_act_tables()

    f32 = mybir.dt.float32
    bf16 = mybir.dt.bfloat16
    AF = mybir.ActivationFunctionType

    nc = bacc.Bacc("TRN2", target_bir_lowering=False, debug=False)

    embT_h = nc.dram_tensor("embT", [E, B], bf16, kind="ExternalInput")
    ker_h = nc.dram_tensor("ker", [E, CS], f32, kind="ExternalInput")
    out_h = nc.dram_tensor("out", [B, CS], f32, kind="ExternalOutput")
    cos_h = nc.dram_tensor("cos", [B, CS], f32, kind="ExternalOutput")
    sums_h = nc.dram_tensor("sums", [128, NB], f32, kind="ExternalOutput")

    embT = embT_h.ap()
    # [p, e, c]: element (e*128+p, c) of the [E, CS] shard
    ker_r = ker_h.ap().rearrange("(e p) c -> p e c", p=128)
    out_r = out_h.ap().rearrange("(b p) c -> p b c", p=128)
    cos_r = cos_h.ap().rearrange("(b p) c -> p b c", p=128)

    with tile.TileContext(nc) as tc, ExitStack() as ctx:
        const_pool = ctx.enter_context(tc.tile_pool(name="const", bufs=1))
        emb_pool = ctx.enter_context(tc.tile_pool(name="emb", bufs=1))
        kb_pool = ctx.enter_context(tc.tile_pool(name="kb", bufs=6))
        sq_pool = ctx.enter_context(tc.tile_pool(name="sq", bufs=3))
        row_pool = ctx.enter_context(tc.tile_pool(name="row", bufs=3))
        bc_pool = ctx.enter_context(tc.tile_pool(name="bc", bufs=3))
        work_pool = ctx.enter_context(tc.tile_pool(name="work", bufs=3))
        exp_pool = ctx.enter_context(tc.tile_pool(name="expp", bufs=3))
        red_pool = ctx.enter_context(tc.tile_pool(name="red", bufs=4))
        acc_pool = ctx.enter_context(tc.tile_pool(name="acc", bufs=1))
        mm_ps = ctx.enter_context(tc.tile_pool(name="mmps", bufs=4, space="PSUM"))
        ss_ps = ctx.enter_context(tc.tile_pool(name="ssps", bufs=3, space="PSUM"))

        ones_col = const_pool.tile([128, 1], bf16, tag="ones_col")
        nc.vector.memset(ones_col[:], 1.0)
        ln_s = const_pool.tile([1, 1], f32, tag="ln_s")
        nc.vector.memset(ln_s[:], math.log(S_SCALE))

        # embT chunks (bf16): emb_tiles[e] = rows e*128..e*128+127, all B cols
        emb_tiles = []
        for e in range(NE):
            t = emb_pool.tile([128, B], bf16, tag=f"embt{e}")
            nc.sync.dma_start(t[:], embT[e * 128:(e + 1) * 128, :])
            emb_tiles.append(t)

        acc = acc_pool.tile([128, NB], f32)
        nc.vector.memset(acc[:], 0.0)

        for ct in range(NCT):
            csl = slice(ct * CT, (ct + 1) * CT)

            # casting DMA: f32 in DRAM -> bf16 in SBUF (SWDGE/gpsimd only)
            kb = kb_pool.tile([128, NE, CT], bf16, tag="kb")
            nc.gpsimd.dma_start(out=kb[:], in_=ker_r[:, :, csl])

            # column sum-of-squares -> [1, CT] psum
            sq = sq_pool.tile([128, NE, CT], bf16, tag="sq")
            nc.vector.tensor_mul(sq[:], kb[:], kb[:])
            ssp = ss_ps.tile([1, CT], f32, tag="ssp")
            for e in range(NE):
                nc.tensor.matmul(
                    ssp[:], lhsT=ones_col[:], rhs=sq[:, e, :],
                    start=(e == 0), stop=(e == NE - 1),
                )

            # r64[c] = 64 / sqrt(ss[c]) = Exp(-0.5*Ln(ss) + ln 64)
            lnrow = row_pool.tile([1, CT], f32, tag="lnrow")
            nc.scalar.activation(lnrow[:], ssp[:], AF.Ln)
            r64row = row_pool.tile([1, CT], f32, tag="r64row")
            nc.scalar.activation(
                r64row[:], lnrow[:], AF.Exp, scale=-0.5, bias=ln_s[:]
            )

            # broadcast the per-column scale to all 128 partitions (GpSimd
            # is otherwise idle; PE rank-1 fp32 matmul costs 4x a bf16 one)
            bc64 = bc_pool.tile([128, CT], f32, tag="bc64")
            nc.gpsimd.partition_broadcast(bc64[:], r64row[:])

            out_ct = work_pool.tile([128, NB, CT], f32, tag="out_ct")
            cos_ct = work_pool.tile([128, NB, CT], f32, tag="cos_ct")
            for b in range(NB):
                mp = mm_ps.tile([128, CT], f32, tag="mp")
                for e in range(NE):
                    nc.tensor.matmul(
                        mp[:],
                        lhsT=emb_tiles[e][:, b * 128:(b + 1) * 128],
                        rhs=kb[:, e, :],
                        start=(e == 0), stop=(e == NE - 1),
                    )
                # out = raw * (64/norm); cos = out/64; acc += sum exp(out)
                nc.vector.tensor_mul(out_ct[:, b, :], mp[:], bc64[:])
                if b % 2 == 0:
                    nc.vector.tensor_scalar_mul(
                        cos_ct[:, b, :], out_ct[:, b, :], 1.0 / S_SCALE
                    )
                else:
                    nc.scalar.mul(cos_ct[:, b, :], out_ct[:, b, :], 1.0 / S_SCALE)
                exp_t = exp_pool.tile([128, CT], f32, tag="exp_t")
                red = red_pool.tile([128, 1], f32, tag="red")
                nc.scalar.activation(
                    exp_t[:], out_ct[:, b, :], AF.Exp, accum_out=red[:]
                )
                nc.vector.tensor_add(acc[:, b:b + 1], acc[:, b:b + 1], red[:])
                nc.sync.dma_start(out_r[:, b, csl], out_ct[:, b, :])
                nc.sync.dma_start(cos_r[:, b, csl], cos_ct[:, b, :])

        nc.sync.dma_start(sums_h.ap()[:], acc[:])

    nc.compile()
    return nc


def _get_nc():
    global _NC_CACHE
    if _NC_CACHE is None:
        _NC_CACHE = _build_nc()
    return _NC_CACHE


def _run_device(embT, ker, trace=False):
    """Run the SPMD kernel. Returns (out [B,C], cos [B,C], sumexp [B], results)."""
    from concourse.bass_utils import run_bass_kernel_spmd

    nc = _get_nc()
    in_maps = []
    for i in range(NCORES):
        shard = np.ascontiguousarray(ker[:, i * CS:(i + 1) * CS])
        in_maps.append({"embT": embT, "ker": shard})

    res = run_bass_kernel_spmd(
        nc, in_maps, core_ids=list(range(NCORES)), trace=trace
    )
    outs = res.results
    out = np.concatenate([outs[i]["out"] for i in range(NCORES)], axis=1)
    cos = np.concatenate([outs[i]["cos"] for i in range(NCORES)], axis=1)
    # sums[i][p, b] = sum_c exp(out[b*128+p, c]) over core i's columns
    sums = np.stack([outs[i]["sums"] for i in range(NCORES)]).sum(axis=0)
    sumexp = sums.T.reshape(-1)  # row r = b*128 + p
    return out, cos, sumexp, res


def _embT_bf16(emb):
    import ml_dtypes

    return np.ascontiguousarray(emb.T.astype(ml_dtypes.bfloat16))


def kernel(embeddings, kernel, label):
    emb = np.ascontiguousarray(np.asarray(embeddings, dtype=np.float32))
    ker = np.ascontiguousarray(np.asarray(kernel, dtype=np.float32))
    lab = np.asarray(label).astype(np.int64)

    out, cos, sumexp, _ = _run_device(_embT_bf16(emb), ker)

    # host-side margin fixup at the 512 label positions + loss
    idx = np.arange(B)
    cc = np.clip(cos[idx, lab].astype(np.float64), -1.0, 1.0)
    sin = np.sqrt(np.maximum(0.0, 1.0 - cc * cc))
    ctm = cc * COS_M - sin * SIN_M
    ctm = np.where(cc - THRESHOLD <= 0.0, cc - MM, ctm)
    new_logit = (S_SCALE * ctm).astype(np.float32)
    old_logit = out[idx, lab].copy()
    out[idx, lab] = new_logit

    se = (
        sumexp.astype(np.float64)
        - np.exp(old_logit.astype(np.float64))
        + np.exp(new_logit.astype(np.float64))
    )
    logZ = np.log(se)
    loss = np.float32(np.mean(logZ - new_logit.astype(np.float64)))
    return loss, out, cos


# revision 13
# speedup vs baseline: 2.3211x; 1.0513x over previous
"""ArcFace multi-core Bass kernel for TRN2 (8 NeuronCores).

Reference computation (see original nn module):
  kernel_norm = kernel / (||kernel||_col + 1e-6)
  cos = clip(emb @ kernel_norm, -1, 1)                       [B, C]
  output = cos.at[i, label[i]].set(cos_theta_m) * 64         [B, C]
  loss = mean(logsumexp(output, 1) - output[i, label[i]])
  returns (loss, output, cos)

Strategy: shard the class dim C=100000 across 8 cores (12500 each).
Each core computes its [512, 12500] slice of output=64*cos and cos, plus
per-row partial sums of exp(64*cos).  The margin fixup only touches the
512 label positions, so it (and the final logsumexp/loss) is done on the
host from the gathered tensors, with an O(B) correction of the exp-sums.

Device pipeline per 500-column tile:
  - casting DMA (SWDGE) loads the f32 kernel tile as bf16
  - square on DVE, column sum-of-squares via matmul with ones (PE)
  - 64/sqrt(ss) == Exp(-0.5*Ln(ss) + ln 64) on ACT; all activations are
    steered into ONE table set (natural_log_exp_and_others) to avoid
    per-tile ACT_TABLE_LOADs
  - per-column scale broadcast to 128 partitions via rank-1 matmul
  - 16 bf16 matmuls emb.T @ ker accumulate [128,500] fp32 PSUM tiles
  - epilogue: out = raw*scale (DVE), cos = out/64 (DVE/ACT split),
    exp row-sums fused into the ACT Exp via accum_out
  - 1 MB batched output DMAs
"""

import math
import os

import numpy as np

B, E, C = 512, 512, 100000
NCORES = 8
CS = C // NCORES  # 12500 columns per core
CT = 500          # column tile (one PSUM bank of fp32)
NCT = CS // CT    # 25
NB = B // 128     # 4 row chunks
NE = E // 128     # 4 contraction chunks

S_SCALE = 64.0
MARGIN = 0.5
COS_M = float(np.cos(MARGIN))
SIN_M = float(np.sin(MARGIN))
MM = float(np.sin(MARGIN) * MARGIN)
THRESHOLD = float(np.cos(np.pi - MARGIN))

_NC_CACHE = None


def _steer_act_tables():
    """Make the act-table chooser put Ln/Exp/Square in ONE set.

    bacc's insert_act_table_loads picks, per activation, the first set
    containing its function - Ln lands in 'natural_log' and Exp in
    'exp_and_others', causing an ACT_TABLE_LOAD ping-pong.  Claim these
    functions only in 'natural_log_exp_and_others' (which really does
    contain all of them) so every activation resolves to that one set.
    """
    from concourse import bacc, mybir

    if getattr(bacc, "_arcface_act_steered", False):
        return
    orig = bacc.get_activation_tables

    def patched(arch):
        tabs = orig(arch)
        combined = "natural_log_exp_and_others"
        steer = {
            mybir.ActivationFunctionType.Ln,
            mybir.ActivationFunctionType.Exp,
            mybir.ActivationFunctionType.Square,
        }
        if combined in tabs and steer <= tabs[combined]:
            for name in tabs:
                if name != combined:
                    tabs[name] = tabs[name] - steer
        return tabs

    bacc.get_activation_tables = patched
    bacc._arcface_act_steered = True


def _build_nc():
    """Build + compile the single-core Bass program (run SPMD on 8 cores)."""
    from contextlib import ExitStack

    import concourse.bass as bass
    import concourse.tile as tile
    from concourse import bacc, mybir

    _steer_act_tables()

    f32 = mybir.dt.float32
    bf16 = mybir.dt.bfloat16
    AF = mybir.ActivationFunctionType

    nc = bacc.Bacc("TRN2", target_bir_lowering=False, debug=False)

    embT_h = nc.dram_tensor("embT", [E, B], bf16, kind="ExternalInput")
    ker_h = nc.dram_tensor("ker", [E, CS], f32, kind="ExternalInput")
    out_h = nc.dram_tensor("out", [B, CS], f32, kind="ExternalOutput")
    cos_h = nc.dram_tensor("cos", [B, CS], f32, kind="ExternalOutput")
    sums_h = nc.dram_tensor("sums", [128, NB], f32, kind="ExternalOutput")

    embT = embT_h.ap()
    # [p, e, c]: element (e*128+p, c) of the [E, CS] shard
    ker_r = ker_h.ap().rearrange("(e p) c -> p e c", p=128)
    out_r = out_h.ap().rearrange("(b p) c -> p b c", p=128)
    cos_r = cos_h.ap().rearrange("(b p) c -> p b c", p=128)

    with tile.TileContext(nc) as tc, ExitStack() as ctx:
        const_pool = ctx.enter_context(tc.tile_pool(name="const", bufs=1))
        emb_pool = ctx.enter_context(tc.tile_pool(name="emb", bufs=1))
        kb_pool = ctx.enter_context(tc.tile_pool(name="kb", bufs=10))
        sq_pool = ctx.enter_context(tc.tile_pool(name="sq", bufs=3))
        row_pool = ctx.enter_context(tc.tile_pool(name="row", bufs=3))
        bc_pool = ctx.enter_context(tc.tile_pool(name="bc", bufs=3))
        work_pool = ctx.enter_context(tc.tile_pool(name="work", bufs=4))
        exp_pool = ctx.enter_context(tc.tile_pool(name="expp", bufs=3))
        red_pool = ctx.enter_context(tc.tile_pool(name="red", bufs=4))
        acc_pool = ctx.enter_context(tc.tile_pool(name="acc", bufs=1))
        mm_ps = ctx.enter_context(tc.tile_pool(name="mmps", bufs=4, space="PSUM"))
        ss_ps = ctx.enter_context(tc.tile_pool(name="ssps", bufs=3, space="PSUM"))

        ones_col = const_pool.tile([128, 1], bf16, tag="ones_col")
        nc.vector.memset(ones_col[:], 1.0)
        ln_s = const_pool.tile([1, 1], f32, tag="ln_s")
        nc.vector.memset(ln_s[:], math.log(S_SCALE))

        # embT chunks (bf16): emb_tiles[e] = rows e*128..e*128+127, all B cols
        emb_tiles = []
        for e in range(NE):
            t = emb_pool.tile([128, B], bf16, tag=f"embt{e}")
            nc.sync.dma_start(t[:], embT[e * 128:(e + 1) * 128, :])
            emb_tiles.append(t)

        acc = acc_pool.tile([128, NB], f32)
        nc.vector.memset(acc[:], 0.0)

        for ct in range(NCT):
            csl = slice(ct * CT, (ct + 1) * CT)

            # casting DMA: f32 in DRAM -> bf16 in SBUF (SWDGE/gpsimd only)
            kb = kb_pool.tile([128, NE, CT], bf16, tag="kb")
            nc.gpsimd.dma_start(out=kb[:], in_=ker_r[:, :, csl])

            # column sum-of-squares -> [1, CT] psum
            sq = sq_pool.tile([128, NE, CT], bf16, tag="sq")
            nc.vector.tensor_mul(sq[:], kb[:], kb[:])
            ssp = ss_ps.tile([1, CT], f32, tag="ssp")
            for e in range(NE):
                nc.tensor.matmul(
                    ssp[:], lhsT=ones_col[:], rhs=sq[:, e, :],
                    start=(e == 0), stop=(e == NE - 1),
                )

            # r64[c] = 64 / sqrt(ss[c]) = Exp(-0.5*Ln(ss) + ln 64)
            lnrow = row_pool.tile([1, CT], f32, tag="lnrow")
            nc.scalar.activation(lnrow[:], ssp[:], AF.Ln)
            r64row = row_pool.tile([1, CT], f32, tag="r64row")
            nc.scalar.activation(
                r64row[:], lnrow[:], AF.Exp, scale=-0.5, bias=ln_s[:]
            )

            # broadcast the per-column scale to all 128 partitions (GpSimd
            # is otherwise idle; PE rank-1 fp32 matmul costs 4x a bf16 one)
            bc64 = bc_pool.tile([128, CT], f32, tag="bc64")
            nc.gpsimd.partition_broadcast(bc64[:], r64row[:])

            out_ct = work_pool.tile([128, NB, CT], f32, tag="out_ct")
            cos_ct = work_pool.tile([128, NB, CT], f32, tag="cos_ct")
            for b in range(NB):
                mp = mm_ps.tile([128, CT], f32, tag="mp")
                for e in range(NE):
                    nc.tensor.matmul(
                        mp[:],
                        lhsT=emb_tiles[e][:, b * 128:(b + 1) * 128],
                        rhs=kb[:, e, :],
                        start=(e == 0), stop=(e == NE - 1),
                    )
                # out = raw * (64/norm); cos = out/64; acc += sum exp(out)
                nc.vector.tensor_mul(out_ct[:, b, :], mp[:], bc64[:])
                if b % 2 == 0:
                    nc.vector.tensor_scalar_mul(
                        cos_ct[:, b, :], out_ct[:, b, :], 1.0 / S_SCALE
                    )
                else:
                    nc.scalar.mul(cos_ct[:, b, :], out_ct[:, b, :], 1.0 / S_SCALE)
                exp_t = exp_pool.tile([128, CT], f32, tag="exp_t")
                red = red_pool.tile([128, 1], f32, tag="red")
                nc.scalar.activation(
                    exp_t[:], out_ct[:, b, :], AF.Exp, accum_out=red[:]
                )
                nc.vector.tensor_add(acc[:, b:b + 1], acc[:, b:b + 1], red[:])
                nc.sync.dma_start(out_r[:, b, csl], out_ct[:, b, :])
                nc.scalar.dma_start(cos_r[:, b, csl], cos_ct[:, b, :])

        nc.sync.dma_start(sums_h.ap()[:], acc[:])

    nc.compile()
    return nc


def _get_nc():
    global _NC_CACHE
    if _NC_CACHE is None:
        _NC_CACHE = _build_nc()
    return _NC_CACHE


def _run_device(embT, ker, trace=False):
    """Run the SPMD kernel. Returns (out [B,C], cos [B,C], sumexp [B], results)."""
    from concourse.bass_utils import run_bass_kernel_spmd

    nc = _get_nc()
    in_maps = []
    for i in range(NCORES):
        shard = np.ascontiguousarray(ker[:, i * CS:(i + 1) * CS])
        in_maps.append({"embT": embT, "ker": shard})

    res = run_bass_kernel_spmd(
        nc, in_maps, core_ids=list(range(NCORES)), trace=trace
    )
    outs = res.results
    out = np.concatenate([outs[i]["out"] for i in range(NCORES)], axis=1)
    cos = np.concatenate([outs[i]["cos"] for i in range(NCORES)], axis=1)
    # sums[i][p, b] = sum_c exp(out[b*128+p, c]) over core i's columns
    sums = np.stack([outs[i]["sums"] for i in range(NCORES)]).sum(axis=0)
    sumexp = sums.T.reshape(-1)  # row r = b*128 + p
    return out, cos, sumexp, res


def _embT_bf16(emb):
    import ml_dtypes

    return np.ascontiguousarray(emb.T.astype(ml_dtypes.bfloat16))


def kernel(embeddings, kernel, label):
    emb = np.ascontiguousarray(np.asarray(embeddings, dtype=np.float32))
    ker = np.ascontiguousarray(np.asarray(kernel, dtype=np.float32))
    lab = np.asarray(label).astype(np.int64)

    out, cos, sumexp, _ = _run_device(_embT_bf16(emb), ker)

    # host-side margin fixup at the 512 label positions + loss
    idx = np.arange(B)
    cc = np.clip(cos[idx, lab].astype(np.float64), -1.0, 1.0)
    sin = np.sqrt(np.maximum(0.0, 1.0 - cc * cc))
    ctm = cc * COS_M - sin * SIN_M
    ctm = np.where(cc - THRESHOLD <= 0.0, cc - MM, ctm)
    new_logit = (S_SCALE * ctm).astype(np.float32)
    old_logit = out[idx, lab].copy()
    out[idx, lab] = new_logit

    se = (
        sumexp.astype(np.float64)
        - np.exp(old_logit.astype(np.float64))
        + np.exp(new_logit.astype(np.float64))
    )
    logZ = np.log(se)
    loss = np.float32(np.mean(logZ - new_logit.astype(np.float64)))
    return loss, out, cos
